# revision 77
# baseline (speedup 1.0000x reference)
# Trainium2 Bass kernel for nn_MultiHeadAttention_85933705658435
#
# Reference: LayerNorm(scale-only) -> QKV projection -> full softmax attention
#            -> output projection.  x:[S,B,E] f32, S=2048 B=2 E=1024, H=16 D=64.
#
# Sharding (8 cores): batch (2-way) x head-group (4-way, 4 heads/core).
#   - W_qkv column-sharded (the core's 4 heads), W_out row-sharded; the 4
#     partial output projections per batch are summed on the host.
#
# Structure (~290us vs the 319us phase-serial baseline).  The key hardware
# facts this design is built around, all measured from ntff profiles:
#   - each hwdge DMA queue sustains only ~106 GB/s (SWDGE ~50), so the
#     front is input-bandwidth-bound: x is pre-rounded to bf16 on the host
#     (4.2MB instead of 8.4MB) and loaded up-front into a persistent tile,
#     split across the scalar + SWDGE queues;
#   - every engine queue is a strict FIFO: an instruction waiting on a slow
#     dependency blocks everything behind it, so LN, evacuations, exps,
#     writes and transposes are each routed/ordered so no stream ever waits
#     behind another phase's dependency chain;
#   - DMA-transpose costs ~9us/MB of queue time with a ~1.1us/instruction
#     floor, so ln bounces through DRAM and is transposed as [512,128]
#     blocks on the sync queue, with the bounce writes spread over the
#     other queues;
#   - the PE is the overall critical resource (~190us busy): V' is computed
#     token-major straight into SBUF (no V bounce), the two heads of each
#     score matmul run concurrently via tile_position row groups, and the
#     appended-ones column of V' yields the softmax denominators for free;
#   - ACT exp (~145us total) is streamed from ~30us in: round 0 and round
#     1-pr0's scores+exp are hoisted into the QKV phase, round-0-pr0's ctx
#     runs in-front (psS 4 + psC 2 + psF 2 = 8 PSUM banks), and each
#     round's pr0 ctx is interleaved into the kc loop so the eP staging
#     slots recycle early for the next round;
#   - fp8e4 exp + DoubleRow ctx was implemented (CTX_FP8/CTX_DR) but fp8
#     quantization of the softmax weights costs ~3% relative error vs the
#     2e-2 budget, so it stays off.

import numpy as np
import ml_dtypes

S, B, E = 2048, 2, 1024
H, D = 16, 64
HPC = 4              # heads per core
NCORES = 8
EPS = 1e-6
FQK = HPC * D        # 256 (per-core Q width = K width = V width)
P = 128
TC = S // P          # 16 token chunks
ECH = E // P         # 8 e-chunks
NSC = 4              # superchunks in the front pipeline
SCT = S // NSC       # 512 tokens per superchunk
QTS = 512            # q-tile size in attention
NQT = S // QTS       # 4
NKP = TC // 2        # 8 key-chunk pairs

CTX_FP8 = False      # fp8e4 exp quantization fails the 2e-2 error budget
CTX_DR = False       # DoubleRow ctx matmuls (requires CTX_FP8)
EXP_SHIFT = -1.6 if CTX_FP8 else 0.0

BF16 = ml_dtypes.bfloat16

_CACHE = {}


def _build_nc():
    from contextlib import ExitStack

    import concourse.bass as bass
    import concourse.tile as tile
    from concourse import bacc, mybir
    from concourse.tile import add_dep_helper

    dt = mybir.dt
    Alu = mybir.AluOpType
    Act = mybir.ActivationFunctionType
    EDT = dt.float8e4 if CTX_FP8 else dt.bfloat16
    DR = mybir.MatmulPerfMode.DoubleRow

    nc = bacc.Bacc(trn_type="TRN2")
    # x arrives pre-rounded to bf16 by the host (halves the dominant input
    # DMA; the LN statistics from bf16-rounded x shift results by well
    # under the bf16 rounding already inherent in the matmul path)
    x_d = nc.dram_tensor("x", (S, E), dt.bfloat16, kind="ExternalInput").ap()
    # wqkv: [E, 3*FQK] = Q | K | V column blocks for this core's 4 heads
    wqkv_d = nc.dram_tensor(
        "wqkv", (E, 3 * FQK), dt.bfloat16, kind="ExternalInput"
    ).ap()
    wo_d = nc.dram_tensor("wo", (FQK, E), dt.bfloat16, kind="ExternalInput").ap()
    out_d = nc.dram_tensor("out", (S, E), dt.float32, kind="ExternalOutput").ap()

    with tile.TileContext(nc) as tc, ExitStack() as ctx:
        singles = ctx.enter_context(tc.tile_pool(name="singles", bufs=1))
        xp = ctx.enter_context(tc.tile_pool(name="xp", bufs=3))
        lnp = ctx.enter_context(tc.tile_pool(name="lnp", bufs=3))
        small = ctx.enter_context(tc.tile_pool(name="small", bufs=4))
        evac = ctx.enter_context(tc.tile_pool(name="evac", bufs=2))
        dram = ctx.enter_context(tc.tile_pool(name="dram", bufs=1, space="DRAM"))

        # persistent SBUF tensors
        lnT = singles.tile([P, ECH, S], dt.bfloat16)          # ln^T, e-chunked
        qkT = singles.tile([P, 4, S], dt.bfloat16)            # fc 0,1: Q^T; 2,3: K^T
        # token-major V (+ ones col at 64) per t-chunk / head; 68 pad so the
        # DoubleRow k-pair stride (4*68) is a multiple of 16 bytes
        Vp = singles.tile([P, TC, HPC, 68], EDT)
        # exp staging: (pr, kcp, head, parity, q); parity = kc&1 so a kcp
        # slice is the [Ki, Ko=2, N] moving operand of the DoubleRow ctx mm
        eP = singles.tile([P, 2, NKP, 2, 2, QTS], EDT)
        ones_dr = singles.tile([P, 2, 1], EDT)                # denominator lhsT
        w_sb = singles.tile([P, ECH, 3 * FQK], dt.bfloat16)
        wo_sb = singles.tile([P, 2, E], dt.bfloat16)
        eps_sb = singles.tile([P, 1], dt.float32)
        ctxn = singles.tile([P, 2, S], dt.bfloat16)           # normalized ctx^T
        ln_dram = dram.tile([S, E], dt.bfloat16)
        rc_dram = dram.tile([NQT * 4, QTS], dt.float32)

        warm = singles.tile([P, 512], dt.bfloat16)
        xt = singles.tile([P, TC, E], dt.bfloat16)            # full x, bf16
        shf_sb = singles.tile([P, 1], dt.float32)
        nc.vector.memset(warm[:], 0.25)
        nc.vector.memset(shf_sb[:], EXP_SHIFT)
        nc.vector.memset(eps_sb[:], EPS)
        # split the weight load so the sync queue head (which delays the
        # first ln writes + transposes) only carries what QKV needs first:
        # K|V columns on sync, Q columns + wo via SWDGE
        wkv_view = wqkv_d.rearrange("(c p) f -> p c f", p=P)
        # only the K columns ride the sync head (0.5MB before sc0's ln
        # writes); Q now, V after the x tail, both on SWDGE
        nc.sync.dma_start(w_sb[:, :, FQK : 2 * FQK], wkv_view[:, :, FQK : 2 * FQK])
        nc.gpsimd.dma_start(w_sb[:, :, 0:FQK], wkv_view[:, :, 0:FQK])

        # ---- front: LN -> transpose -> QKV ------------------------------
        # Emission order is engineered around the per-engine FIFO queues:
        #   - ALL LN chunks first, so the DVE queue is a clean LN pipeline
        #     (an evacuation emitted mid-LN would stall later LN chunks
        #     behind the whole transpose->QKV chain)
        #   - transposes per superchunk on the sync queue (dispatch-bound,
        #     ~9us/MB: they get the queue to themselves)
        #   - ln bounce writes on the SWDGE queue, x loads on the scalar
        #     queue, each free-running
        #   - QKV per superchunk afterwards, with round-0 scores+exp
        #     hoisted between superchunk groups so ACT starts ~25us in
        # scores psum: [h_even | h_odd] per kc, 2 banks each, double-buffered.
        # psS(4) + psC(2) + psF(2) = 8 banks during the front; psF's 2 are
        # recycled into the outproj pool afterwards.
        psS = ctx.enter_context(tc.tile_pool(name="psS", bufs=2, space="PSUM"))
        psC = ctx.enter_context(tc.tile_pool(name="psC", bufs=1, space="PSUM"))

        def ctx_open(pr):
            # ctx accumulators [65, QTS] per head: row 64 accumulates the
            # softmax denominator via the ones column of V'
            return [
                psC.tile([65, QTS], dt.float32, tag=f"cps{h}", name=f"cps{h}")
                for h in range(2)
            ]

        def ctx_kcp(pr, kcp, cps):
            for h in range(2):
                hh = pr * 2 + h
                if CTX_DR:
                    nc.tensor.matmul(
                        cps[h][:],
                        Vp[:, 2 * kcp : 2 * kcp + 2, hh, 0:65],
                        eP[:, pr, kcp, h, :, :],
                        start=(kcp == 0), stop=(kcp == NKP - 1),
                        perf_mode=DR,
                    )
                else:
                    for par in range(2):
                        nc.tensor.matmul(
                            cps[h][:],
                            Vp[:, 2 * kcp + par, hh, 0:65],
                            eP[:, pr, kcp, h, par, :],
                            start=(kcp == 0 and par == 0),
                            stop=(kcp == NKP - 1 and par == 1),
                        )

        def scores_exp(pr, qt, kc):
            q0 = qt * QTS
            k0 = kc * P
            kcp, par = divmod(kc, 2)
            sq = psS.tile([P, 2 * QTS], dt.float32, tag="sq", name="sq")
            nc.tensor.matmul(
                sq[:, 0:QTS],
                qkT[0:64, 2 + pr, k0 : k0 + P],
                qkT[0:64, pr, q0 : q0 + QTS],
                start=True, stop=True, tile_position=(0, 0),
            )
            nc.tensor.matmul(
                sq[:, QTS : 2 * QTS],
                qkT[64:128, 2 + pr, k0 : k0 + P],
                qkT[64:128, pr, q0 : q0 + QTS],
                start=True, stop=True, tile_position=(64, 0),
            )
            # exp(s - C): softmax is shift-invariant (the ones-column
            # denominator uses the same shifted values).  The shift keeps
            # the heavy score tail under fp8e4m3's 448 max; the underflow
            # of tiny weights costs <0.2% of the denominator mass.
            nc.scalar.activation(
                eP[:, pr, kcp, :, par, :],
                sq[:].rearrange("p (h q) -> p h q", h=2),
                Act.Exp,
                bias=shf_sb[:],
            )

        with tc.tile_pool(name="psF", bufs=2, space="PSUM") as psF:
            # HAM warmup on a local dummy tile (no DMA dependency: the PE
            # starts churning ~1us in, not after the 14us w_sb load)
            wps = psF.tile([P, 512], dt.float32, tag="qkv", name="wps")
            for _ in range(36):
                nc.tensor.matmul(
                    wps[:], warm[:, 0:P], warm[:],
                    start=True, stop=True,
                )

            # all x loads dispatched up-front into a persistent tile: the
            # scalar hwdge queue (~106 GB/s) carries most, the SWDGE queue
            # the tail, and nothing ever queues ahead of them
            for t in range(TC):
                q = nc.scalar if t < 12 else nc.gpsimd
                q.dma_start(xt[:, t, :], x_d[t * P : (t + 1) * P, :])
            # V columns after the x tail on SWDGE: V matmuls don't gate the
            # critical path (scores need only K and Q)
            nc.gpsimd.dma_start(
                w_sb[:, :, 2 * FQK :], wkv_view[:, :, 2 * FQK :]
            )

            def ln_sc(sc):
                t0, t1 = sc * (TC // NSC), (sc + 1) * (TC // NSC)
                ln_writes = []
                for t in range(t0, t1):
                    xb = xt[:, t, :]
                    st = small.tile([P, 2, 6], dt.float32, tag="st")
                    nc.vector.bn_stats(st[:, 0, :], xb[:, 0:512])
                    nc.vector.bn_stats(st[:, 1, :], xb[:, 512:1024])
                    mv = small.tile([P, 2], dt.float32, tag="mv")
                    nc.vector.bn_aggr(mv[:], st[:])
                    sd = small.tile([P, 1], dt.float32, tag="sd")
                    nc.scalar.activation(sd[:], mv[:, 1:2], Act.Sqrt, bias=eps_sb[:])
                    rs = small.tile([P, 1], dt.float32, tag="rs")
                    nc.vector.reciprocal(rs[:], sd[:])
                    lnb = lnp.tile([P, E], dt.bfloat16, tag="lnb", bufs=3)
                    if 4 <= t < 12:
                        # ACT is idle mid-front: offload the normalize for
                        # the middle chunks: ln = x*rs + (-mu*rs).  The
                        # first chunks stay on DVE (shortest critical path
                        # to the first transposes) and sc3's stay on DVE
                        # (their ACT ops would block the hoisted exps in
                        # the ACT FIFO while waiting on the slow x tail).
                        nb = small.tile([P, 1], dt.float32, tag="nb")
                        nc.vector.tensor_tensor(nb[:], mv[:, 0:1], rs[:], Alu.mult)
                        nc.vector.tensor_scalar_mul(nb[:], nb[:], -1.0)
                        nc.scalar.activation(
                            lnb[:], xb, Act.Identity, bias=nb[:], scale=rs[:]
                        )
                    else:
                        nc.vector.tensor_scalar(
                            lnb[:], xb, mv[:, 0:1], rs[:], Alu.subtract, Alu.mult
                        )
                    # bounce via DRAM: DMA-transpose pays ~1.1us fixed per
                    # instruction, so SBUF-side [128,128] transposes lose;
                    # [512,128] DRAM-side reads amortize it.  The writes
                    # round-robin over all three queues: the sync queue's
                    # ~106 GB/s must mostly go to the transposes.
                    # sc0's writes all on sync (free after the small K-column
                    # load; the scalar queue is busy with x until ~30us)
                    if sc == 0:
                        wq = nc.sync
                    else:
                        wq = (nc.scalar, nc.sync, nc.scalar, nc.gpsimd)[t % 4]
                    ln_writes.append(
                        wq.dma_start(ln_dram[t * P : (t + 1) * P, :], lnb[:])
                    )
                for c in range(ECH):
                    # late superchunks' transposes ride the scalar queue
                    # (idle once the x loads finish ~30us in); sync alone
                    # cannot move writes + 4.2MB of transposes in time
                    tq = nc.sync
                    tp = tq.dma_start_transpose(
                        lnT[:, c, sc * SCT : (sc + 1) * SCT],
                        ln_dram[sc * SCT : (sc + 1) * SCT, c * P : (c + 1) * P],
                    )
                    for wi in ln_writes:
                        add_dep_helper(tp.ins, wi.ins, True, "lnT RAW via ln_dram")

            def qkv_kq(sc):
                # K then Q, feature-major, weights-stationary
                for fc in (2, 3, 0, 1):
                    ps = psF.tile([P, 512], dt.float32, tag="qkv")
                    for ec in range(ECH):
                        nc.tensor.matmul(
                            ps[:],
                            w_sb[:, ec, fc * P : (fc + 1) * P],
                            lnT[:, ec, sc * SCT : (sc + 1) * SCT],
                            start=(ec == 0),
                            stop=(ec == ECH - 1),
                        )
                    nc.vector.tensor_copy(qkT[:, fc, sc * SCT : (sc + 1) * SCT], ps[:])
            def qkv_v(sc):
                # V token-major: lnT chunk stationary, V weight cols moving.
                # Shares the "qkv" psum tag (2-deep rotation) so the whole
                # front fits 2 PSUM banks, freeing 2 for the early ctx pool.
                t0, t1 = sc * (TC // NSC), (sc + 1) * (TC // NSC)
                for t in range(t0, t1):
                    vps = psF.tile([P, 512], dt.float32, tag="qkv")
                    for ec in range(ECH):
                        nc.tensor.matmul(
                            vps[:, 0:FQK],
                            lnT[:, ec, t * P : (t + 1) * P],
                            w_sb[:, ec, 2 * FQK : 3 * FQK],
                            start=(ec == 0),
                            stop=(ec == ECH - 1),
                        )
                    nc.vector.tensor_copy(
                        Vp[:, t, :, 0:64],
                        vps[:, 0:FQK].rearrange("p (h d) -> p h d", d=64),
                    )

            # Emission order: LN for sc0-2 first (x via the fast scalar
            # queue), then QKV + hoisted scores/exp (round 0 fully, and
            # q-tile 1's pr0 stream), then LN(sc3) (its x rides the slow
            # SWDGE queue) -- if LN(sc3)'s sqrt were emitted before the
            # hoisted exps, it would block them in the ACT FIFO until sc3's
            # x arrives.  Round 0 pr0's ctx runs in-front too (psC is open),
            # freeing its eP slots for q-tile 1's hoisted exps.
            for sc in range(3):
                ln_sc(sc)
            # Vp init here, AFTER the LN emission: on the DVE FIFO these
            # big memsets would otherwise run before the first LN stats and
            # delay the whole front by ~5us
            nc.vector.memset(Vp[:], 0.0)
            nc.vector.memset(Vp[:, :, :, 64:65], 1.0)
            nc.vector.memset(ones_dr[:], 1.0)
            cpsA0 = None
            for sc in range(3):
                qkv_kq(sc)
                if sc == 0:
                    cpsA0 = ctx_open(0)
                # scores/exp BEFORE the V matmuls: sc0's V waits for the
                # late V-column load and must not block them in the PE FIFO
                for kc in range(4 * sc, 4 * sc + 4):
                    scores_exp(0, 0, kc)
                    scores_exp(1, 0, kc)
                qkv_v(sc)
                for kc in range(4 * sc + 1, 4 * sc + 4, 2):
                    ctx_kcp(0, kc // 2, cpsA0)
                if sc >= 1:
                    # q-tile 1 (tokens 512-1023: Q from sc1) pr0 stream
                    for kc in range(4 * (sc - 1), 4 * sc):
                        scores_exp(0, 1, kc)
                if sc == 2:
                    for kc in range(8, 12):
                        scores_exp(0, 1, kc)
            ln_sc(3)
            qkv_kq(3)
            qkv_v(3)
            # wo rides SWDGE after the x tail; needed only by outproj
            nc.gpsimd.dma_start(wo_sb[:], wo_d.rearrange("(c p) e -> p c e", p=P))

        # ---- attention: per q-tile, both head-pairs' exp interleaved ------
        ctx2 = ExitStack()
        with ctx2:
            psO = ctx2.enter_context(tc.tile_pool(name="psO", bufs=1, space="PSUM"))

            def normalize(pr, qt, cps):
                # the last round's normalize gates the final out-projection
                # directly; route its DMA chain through the scalar queue
                # (idle once the exps are done) instead of queueing behind
                # the out-writes on sync
                nq = nc.scalar if qt == NQT - 1 else nc.sync
                # ctx^T[d,q] / denom[q]; denom is ctx row 64.  reciprocal of
                # a [1,512] row is ~3us on one DVE lane, so reshape to
                # [128,4] via a small DMA, recip, then broadcast across 64
                # partitions with a step-0 DMA read from DRAM.
                q0 = qt * QTS
                ctxu = []
                for h in range(2):
                    cu = evac.tile([65, QTS], dt.float32, tag=f"ctxu{h}",
                                   name=f"ctxu{h}")
                    nc.vector.tensor_copy(cu[:], cps[h][:])
                    ctxu.append(cu)
                for h in range(2):
                    dnp = small.tile([P, QTS // P], dt.float32, tag="dnp")
                    nq.dma_start(dnp[:], ctxu[h][64:65, :])
                    rcp = small.tile([P, QTS // P], dt.float32, tag="rcp")
                    nc.vector.reciprocal(rcp[:], dnp[:])
                    slot = (qt * 2 + pr) * 2 + h
                    rc_row = rc_dram[slot : slot + 1, :]
                    wr = nq.dma_start(rc_row, rcp[:])
                    bcs = evac.tile([64, QTS], dt.float32, tag="bcs")
                    rc_bcast = bass.AP(
                        tensor=rc_row.tensor,
                        offset=rc_row.offset,
                        ap=[[0, 64]] + list(rc_row.ap[1:]),
                    )
                    rd = nq.dma_start(bcs[:], rc_bcast)
                    add_dep_helper(rd.ins, wr.ins, True, "recip RAW via dram")
                    if h == 0:
                        nc.vector.tensor_tensor(
                            ctxn[0:64, pr, q0 : q0 + QTS],
                            ctxu[0][0:64, :], bcs[:], Alu.mult,
                        )
                    else:
                        tmpn = evac.tile([64, QTS], dt.bfloat16, tag="tmpn")
                        nc.vector.tensor_tensor(
                            tmpn[:], ctxu[1][0:64, :], bcs[:], Alu.mult
                        )
                        # partition shift 0-63 -> 64-127 via SBUF-SBUF DMA
                        nq.dma_start(ctxn[64:128, pr, q0 : q0 + QTS], tmpn[:])

            def outproj(qt):
                for t in range(qt * (QTS // P), (qt + 1) * (QTS // P)):
                    po = psO.tile([P, E], dt.float32, tag="po")
                    for et in range(2):
                        for pr in range(2):
                            nc.tensor.matmul(
                                po[:, et * 512 : (et + 1) * 512],
                                ctxn[:, pr, t * P : (t + 1) * P],
                                wo_sb[:, pr, et * 512 : (et + 1) * 512],
                                start=(pr == 0), stop=(pr == 1),
                            )
                    ob = evac.tile([P, E], dt.float32, tag="ob", bufs=2)
                    nc.vector.tensor_copy(ob[:], po[:])
                    # output writes alternate sync/SWDGE (NOT the scalar
                    # queue, which runs the exps and would stall behind the
                    # normalize chain these writes depend on); the final
                    # q-tile drains on sync only (SWDGE is too slow to
                    # drain the tail)
                    if t % 2 == 0 or t >= TC - 4:
                        nc.sync.dma_start(out_d[t * P : (t + 1) * P, :], ob[:])
                    else:
                        nc.gpsimd.dma_start(out_d[t * P : (t + 1) * P, :], ob[:])

            # outproj(qt) is EMITTED a few kc into round qt+1: the PE queue
            # is a static FIFO, and outproj's ctxn dependency sits behind a
            # multi-DMA normalize chain -- placing it before the next
            # round's scores would stall the scores and starve ACT
            # Round structure: pr0's ctx matmuls are interleaved into the kc
            # loop (each kcp right after its exps land), so pr0's eP slots
            # free mid-round and the NEXT round's exp stream starts without
            # waiting for this round to finish.  pr1's ctx runs compactly at
            # the round end (~4us of PE), gated only by pr0's evacuation
            # (shared cps tags).  outproj(qt) is emitted a few kc into round
            # qt+1 so its slow normalize dependency can't stall the scores
            # in the static PE FIFO.
            pend = None
            pre = 0
            for qt in range(NQT):
                # round 0 entirely and round 1's pr0 kc0-11 were hoisted
                cpsA = cpsA0 if qt == 0 else ctx_open(0)
                for kc in range(12 if qt == 0 else 0, TC):
                    if (qt != 1 or kc >= 12) and kc >= pre:
                        scores_exp(0, qt, kc)
                    scores_exp(1, qt, kc)
                    if kc % 2 == 1:
                        ctx_kcp(0, kc // 2, cpsA)
                    if kc == 5 and pend is not None:
                        outproj(pend)
                        pend = None
                # pre-emit the next round's first pr0 scores+exp: without
                # this, ACT starves ~6us at each boundary behind the PE
                # FIFO's [evac(pr0) -> ctx(pr1) x16] chain.  pr1 cannot be
                # pre-emitted: its eP slots' old values are read by the
                # ctx(pr1) block emitted below, and Tile dependencies
                # follow emission order.
                pre = 0
                if qt < NQT - 1 and qt + 1 != 1:
                    for kc in range(6):
                        scores_exp(0, qt + 1, kc)
                    pre = 6
                normalize(0, qt, cpsA)
                cpsB = ctx_open(1)
                for kcp in range(NKP):
                    ctx_kcp(1, kcp, cpsB)
                normalize(1, qt, cpsB)
                pend = qt
            outproj(pend)

    nc.compile()
    return nc


def make_in_maps(x, ln_scale, w_qkv, w_out):
    w = (np.asarray(w_qkv, np.float32) * np.asarray(ln_scale, np.float32)[:, None])
    wo = np.asarray(w_out, np.float32)
    in_maps = []
    for c in range(NCORES):
        b, g = divmod(c, 4)
        h0 = g * HPC
        wq = w[:, h0 * D : (h0 + HPC) * D]
        wk = w[:, H * D + h0 * D : H * D + (h0 + HPC) * D]
        wv = w[:, 2 * H * D + h0 * D : 2 * H * D + (h0 + HPC) * D]
        in_maps.append(
            {
                "x": np.ascontiguousarray(
                    np.asarray(x, np.float32)[:, b, :]
                ).astype(BF16),
                "wqkv": np.ascontiguousarray(
                    np.concatenate([wq, wk, wv], axis=1)
                ).astype(BF16),
                "wo": np.ascontiguousarray(
                    wo[h0 * D : (h0 + HPC) * D, :]
                ).astype(BF16),
            }
        )
    return in_maps


def get_nc():
    if "nc" not in _CACHE:
        _CACHE["nc"] = _build_nc()
    return _CACHE["nc"]


def assemble(results):
    out = np.empty((S, B, E), np.float32)
    for b in range(B):
        acc = results[4 * b]["out"].astype(np.float32).copy()
        for g in range(1, 4):
            acc += results[4 * b + g]["out"]
        out[:, b, :] = acc
    return out


def kernel(x, ln_scale, w_qkv, w_out):
    from concourse.bass_utils import run_bass_kernel_spmd

    nc = get_nc()
    in_maps = make_in_maps(x, ln_scale, w_qkv, w_out)
    res = run_bass_kernel_spmd(nc, in_maps, core_ids=list(range(NCORES)))
    return assemble(res.results)
